# revision 4
# baseline (speedup 1.0000x reference)
"""Multi-head attention (B=4, S=2048, D=1024, H=16) on 8 Trainium2 NeuronCores.

Sharding: core c handles batch c//2 and head-group c%2 (8 heads = 512 dims of
the per-head concat). Each core computes its q/k/v projections (tensor
parallel over heads), attention for its 8 heads, and a partial output
projection over its 512 concat dims; the host sums the two partials per batch.

Device dataflow (per core, all matmuls bf16 = full-rate PE with FWL weight
loads and background-buffer preload; fp32 PSUM accumulation):
  - prologue projects only kT[dc0] and qT[qt0][dc0]; attention starts ~10us
    in. All remaining projection work (v, kT dc1-3, qT chunks, out-proj)
    streams through a background generator queue interleaved between the
    attention matmuls, paced so the ACT-bound exp pipeline never starves.
  - scores computed transposed S^T[k, q]; the two heads of a pair are
    row-packed (tile rows 0/64) so they run concurrently on the PE.
  - exp(scale*s) is a single ACT op per k-chunk over both heads (the mask is
    all-ones per the problem spec; a numpy fallback handles anything else).
  - ctx^T = [V | 1]^T @ P^T accumulated over k-chunks; row 64 of the psum is
    the softmax denominator (flash-style deferred normalization).
  - normalization: reciprocal of the denominator row on DVE, partition
    broadcast on GpSimd, one DVE multiply per head into ctx^T.
  - out^T partial = ctx_cat^T chunks @ Wo^T slices, streamed to DRAM.

Host epilogue: out[b] = partial[2b] + partial[2b+1] + (Wo @ bv + bo); the
value bias commutes with softmax (rows sum to 1) so it is exact. The key bias
is softmax-invariant (constant per query) and is still applied on-device for
exactness; so is the query bias.
"""

import sys

sys.path.insert(0, "/opt/trn_rl_repo")

import numpy as np

import concourse.bacc as bacc
import concourse.mybir as mybir
import concourse.tile as tile
from concourse.bass_utils import run_bass_kernel_spmd

f32 = mybir.dt.float32
bf16 = mybir.dt.bfloat16
AF = mybir.ActivationFunctionType

B, S, E, H = 4, 2048, 1024, 16
DH = E // H  # 64
G = E // 2  # 512 dims per core (8 heads)
HL = H // 2  # heads per core
EC = E // 128  # 8 e-chunks (projection contraction)
DC = G // 128  # 4 head-pairs per core
QT = S // 512  # 4 q-tiles
KC = S // 128  # 16 k-chunks
GC = G // 128  # 4 chunks of the local concat dim (out-proj contraction)
SCALE = 1.0 / np.sqrt(np.float64(E))

_NC = None


def _build_program():
    nc = bacc.Bacc("TRN2", target_bir_lowering=False, debug=False, num_devices=8)

    xqT = nc.dram_tensor("xqT", [E, S], bf16, kind="ExternalInput").ap()
    xkT = nc.dram_tensor("xkT", [E, S], bf16, kind="ExternalInput").ap()
    xvT = nc.dram_tensor("xvT", [E, S], bf16, kind="ExternalInput").ap()
    wqT = nc.dram_tensor("wqT", [E, G], bf16, kind="ExternalInput").ap()
    wkT = nc.dram_tensor("wkT", [E, G], bf16, kind="ExternalInput").ap()
    wvT = nc.dram_tensor("wvT", [E, G], bf16, kind="ExternalInput").ap()
    woT = nc.dram_tensor("woT", [G, E], bf16, kind="ExternalInput").ap()
    bqd = nc.dram_tensor("bqd", [128, DC], f32, kind="ExternalInput").ap()
    bkd = nc.dram_tensor("bkd", [128, DC], f32, kind="ExternalInput").ap()
    out = nc.dram_tensor("out", [E, S], f32, kind="ExternalOutput").ap()  # transposed

    with tile.TileContext(nc) as tc:
        with (
            tc.tile_pool(name="weights", bufs=1) as wpool,
            tc.tile_pool(name="persist", bufs=1) as ppool,
            tc.tile_pool(name="xkpool", bufs=4) as xkpool,
            tc.tile_pool(name="xqstream", bufs=2) as xqstream,
            tc.tile_pool(name="vstream", bufs=2) as vstream,
            tc.tile_pool(name="qtile", bufs=2) as qpool,
            tc.tile_pool(name="ctxp", bufs=1) as ctxp,
            tc.tile_pool(name="exp", bufs=8) as epool,
            tc.tile_pool(name="norm", bufs=3) as npool,
            tc.tile_pool(name="outsb", bufs=4) as opool,
            tc.tile_pool(name="s_psum", bufs=2, space="PSUM") as s_psum,
            tc.tile_pool(name="c_psum", bufs=4, space="PSUM") as c_psum,
        ):
            kT_sb = ppool.tile([128, DC, S], bf16)
            v_sb = ppool.tile([128, KC, HL, DH + 1], bf16)
            ctxT_sb = ctxp.tile([128, DC, S], bf16)
            wq_sb = wpool.tile([128, EC, G], bf16)
            wk_sb = wpool.tile([128, EC, G], bf16)
            wv_sb = wpool.tile([128, EC, G], bf16)
            wo_sb = wpool.tile([128, GC, E], bf16)
            bq_sb = wpool.tile([128, DC], f32)
            bk_sb = wpool.tile([128, DC], f32)
            actwarm = wpool.tile([1, 8], f32)

            def xstream(pool, src, lo, eng, tag="xstream"):
                t = pool.tile([128, EC, 512], bf16, tag=tag)
                ap = src[:, lo : lo + 512].rearrange("(ec p) s -> p ec s", p=128)
                eng.dma_start(t[:], ap)
                return t

            # ones column for the denominator fusion (cols 0..DH-1 of each
            # head block are written by the v-projection before any read)
            nc.gpsimd.memset(v_sb[:, :, :, DH : DH + 1], 1.0)
            # pull the exp table load off the attention critical path
            nc.gpsimd.memset(actwarm[:], 0.0)
            nc.scalar.activation(actwarm[:], actwarm[:], AF.Exp)

            # -------- prologue DMAs, split across queues for latency --------
            # sync queue: key-projection path
            nc.sync.dma_start(wk_sb[:], wkT.rearrange("(ec p) g -> p ec g", p=128))
            xk_ts = [xstream(xkpool, xkT, st * 512, nc.sync, tag="xk") for st in range(QT)]
            # gpsimd queue: query-projection path (+ tiny biases)
            nc.gpsimd.dma_start(bq_sb[:], bqd)
            nc.gpsimd.dma_start(bk_sb[:], bkd)
            xq_ts = {0: xstream(xqstream, xqT, 0, nc.gpsimd, tag="xq")}
            nc.gpsimd.dma_start(wq_sb[:], wqT.rearrange("(ec p) g -> p ec g", p=128))
            nc.gpsimd.dma_start(wo_sb[:], woT.rearrange("(gc p) e -> p gc e", p=128))
            # scalar queue: value-projection path
            nc.scalar.dma_start(wv_sb[:], wvT.rearrange("(ec p) g -> p ec g", p=128))
            xv_ts = {
                0: xstream(vstream, xvT, 0, nc.scalar, tag="xv"),
                1: xstream(vstream, xvT, 512, nc.scalar, tag="xv"),
            }

            # ---------------- projection generators ----------------
            def kproj_steps(dc):
                """kT projection for one head-pair, all 4 s-tiles."""
                for st in range(QT):
                    ps = c_psum.tile([128, 512], f32, tag="ctx", name=f"kp{dc}_{st}")
                    for ec in range(EC):
                        nc.tensor.matmul(
                            ps[:],
                            lhsT=wk_sb[:, ec, dc * 128 : (dc + 1) * 128],
                            rhs=xk_ts[st][:, ec, :],
                            start=(ec == 0),
                            stop=(ec == EC - 1),
                        )
                        if ec % 2 == 1:
                            yield
                    nc.vector.tensor_add(
                        out=kT_sb[:, dc, st * 512 : (st + 1) * 512],
                        in0=ps[:],
                        in1=bk_sb[:, dc : dc + 1].to_broadcast((128, 512)),
                    )

            def vproj_steps(sc):
                """v projection for one 128-key chunk (all local heads)."""
                sg, sci = sc // 4, sc % 4
                if sci == 0 and sg + 2 < QT and sg + 2 not in xv_ts:
                    xv_ts[sg + 2] = xstream(vstream, xvT, (sg + 2) * 512, nc.scalar, tag="xv")
                ps = c_psum.tile([128, 512], f32, tag="ctx", name=f"vp{sc}")
                for ec in range(EC):
                    nc.tensor.matmul(
                        ps[:, :G],
                        lhsT=xv_ts[sg][:, ec, sci * 128 : (sci + 1) * 128],
                        rhs=wv_sb[:, ec, :],
                        start=(ec == 0),
                        stop=(ec == EC - 1),
                    )
                    if ec % 2 == 1:
                        yield
                nc.vector.tensor_copy(
                    out=v_sb[:, sc, :, 0:DH],
                    in_=ps[:, :G].rearrange("p (h d) -> p h d", h=HL),
                )

            qT_ts = {}

            def qproj_steps(qt, dc):
                """one dc-chunk of the qT projection."""
                if dc == 0:
                    qT_ts[qt] = qpool.tile(
                        [128, DC, 512], bf16, tag="qT", name=f"qT{qt}"
                    )
                    if qt not in xq_ts:
                        xq_ts[qt] = xstream(xqstream, xqT, qt * 512, nc.sync, tag="xq")
                qT_t = qT_ts[qt]
                ps = c_psum.tile([128, 512], f32, tag="ctx", name=f"qp{qt}_{dc}")
                for ec in range(EC):
                    nc.tensor.matmul(
                        ps[:],
                        lhsT=wq_sb[:, ec, dc * 128 : (dc + 1) * 128],
                        rhs=xq_ts[qt][:, ec, :],
                        start=(ec == 0),
                        stop=(ec == EC - 1),
                    )
                    if ec % 2 == 1:
                        yield
                nc.vector.tensor_add(
                    out=qT_t[:, dc, :],
                    in0=ps[:],
                    in1=bq_sb[:, dc : dc + 1].to_broadcast((128, 512)),
                )

            def outproj_steps(st, ec):
                """one ec-chunk of the transposed output projection."""
                ps = c_psum.tile([128, 512], f32, tag="ctx", name=f"op{st}_{ec}")
                for gc in range(GC):
                    nc.tensor.matmul(
                        ps[:],
                        lhsT=wo_sb[:, gc, ec * 128 : (ec + 1) * 128],
                        rhs=ctxT_sb[:, gc, st * 512 : (st + 1) * 512],
                        start=(gc == 0),
                        stop=(gc == GC - 1),
                    )
                    if gc % 2 == 1:
                        yield
                o_sb = opool.tile([128, 512], f32, tag="osb")
                nc.vector.tensor_copy(out=o_sb[:], in_=ps[:])
                nc.sync.dma_start(
                    out[ec * 128 : (ec + 1) * 128, st * 512 : (st + 1) * 512],
                    o_sb[:],
                )

            # -------- background work queue (persists across hp/qt) --------
            bg = []  # list of (key, generator)
            done_keys = set()

            def drive(n=1):
                while n > 0 and bg:
                    try:
                        next(bg[0][1])
                        n -= 1
                    except StopIteration:
                        done_keys.add(bg[0][0])
                        bg.pop(0)

            def drain_until(key):
                while key not in done_keys and bg:
                    drive(1)

            # -------- prologue compute: kT[dc0] and qT[qt0][dc0] --------
            for _ in kproj_steps(0):
                pass
            done_keys.add(("kT", 0))
            for _ in qproj_steps(0, 0):
                pass
            done_keys.add(("qT", 0, 0))

            for sc in range(KC):
                bg.append((("v", sc), vproj_steps(sc)))
            for dc in range(1, DC):
                bg.append((("kT", dc), kproj_steps(dc)))
                bg.append((("qT", 0, dc), qproj_steps(0, dc)))

            # ---------------- attention main loop ----------------
            for qt in range(QT):
                q0 = qt * 512
                for hp in range(DC):
                    if qt < QT - 1:
                        bg.append((("qT", qt + 1, hp), qproj_steps(qt + 1, hp)))
                    if qt > 0:
                        bg.append((("op", qt - 1, 2 * hp), outproj_steps(qt - 1, 2 * hp)))
                        bg.append(
                            (("op", qt - 1, 2 * hp + 1), outproj_steps(qt - 1, 2 * hp + 1))
                        )
                    drain_until(("kT", hp))
                    drain_until(("qT", qt, hp))
                    qT_t = qT_ts[qt]
                    ctx0 = c_psum.tile([128, 512], f32, tag="ctx", name=f"c0_{qt}_{hp}")
                    ctx1 = c_psum.tile([128, 512], f32, tag="ctx", name=f"c1_{qt}_{hp}")
                    # software-pipelined: ctx(kc-1) and background work are
                    # emitted BEFORE the scores pair of kc so the scheduler
                    # keeps the two row-packed scores matmuls adjacent
                    pend = [None]

                    def ctx_pair(kc):
                        drain_until(("v", kc))
                        e = pend[0]
                        nc.tensor.matmul(
                            ctx0[0 : DH + 1, :],
                            lhsT=v_sb[:, kc, 2 * hp, :],
                            rhs=e[:, 0:512],
                            start=(kc == 0),
                            stop=(kc == KC - 1),
                        )
                        nc.tensor.matmul(
                            ctx1[0 : DH + 1, :],
                            lhsT=v_sb[:, kc, 2 * hp + 1, :],
                            rhs=e[:, 512:1024],
                            start=(kc == 0),
                            stop=(kc == KC - 1),
                        )

                    for kc in range(KC):
                        k0 = kc * 128
                        if kc > 0:
                            ctx_pair(kc - 1)
                        drive(1)
                        sp = s_psum.tile([128, 1024], f32, tag="sp")
                        nc.tensor.matmul(
                            sp[:, 0:512],
                            lhsT=kT_sb[0:64, hp, k0 : k0 + 128],
                            rhs=qT_t[0:64, hp, :],
                            start=True,
                            stop=True,
                        )
                        nc.tensor.matmul(
                            sp[:, 512:1024],
                            lhsT=kT_sb[64:128, hp, k0 : k0 + 128],
                            rhs=qT_t[64:128, hp, :],
                            start=True,
                            stop=True,
                        )
                        e = epool.tile([128, 1024], bf16, tag="exp")
                        nc.scalar.activation(e[:], sp[:], AF.Exp, scale=float(SCALE))
                        pend[0] = e
                    ctx_pair(KC - 1)
                    # evacuate psum fast, then normalize in SBUF
                    for hq, cpsum in ((0, ctx0), (1, ctx1)):
                        pb = 64 * hq
                        qs = slice(q0, q0 + 512)
                        nc.vector.tensor_copy(
                            out=ctxT_sb[pb : pb + 64, hp, qs], in_=cpsum[0:DH, :]
                        )
                        den = npool.tile([1, 512], f32, tag="den")
                        nc.vector.tensor_copy(out=den[:], in_=cpsum[DH : DH + 1, :])
                        rec = npool.tile([1, 512], f32, tag="rec")
                        nc.vector.reciprocal_approx_fast(rec[:], den[:])
                        rb = npool.tile([128, 512], f32, tag="rb")
                        nc.gpsimd.partition_broadcast(rb[:], rec[:])
                        nc.vector.tensor_mul(
                            out=ctxT_sb[pb : pb + 64, hp, qs],
                            in0=ctxT_sb[pb : pb + 64, hp, qs],
                            in1=rb[pb : pb + 64, :],
                        )

            # drain any background leftovers, then the last q-tile's out-proj
            while bg:
                drive(1)
            for ec in range(EC):
                for _ in outproj_steps(QT - 1, ec):
                    pass

    nc.compile()
    return nc


def _prep_core_inputs(query, key, value, Wq, bq, Wk, bk, Wv, Wo):
    """Per-core input maps: core c -> batch c//2, head-group c%2."""
    import ml_dtypes

    f = ml_dtypes.bfloat16
    maps = []
    for c in range(8):
        b, g = c // 2, c % 2
        lo = g * G
        maps.append(
            {
                "xqT": np.ascontiguousarray(query[b].T).astype(f, copy=False),
                "xkT": np.ascontiguousarray(key[b].T).astype(f, copy=False),
                "xvT": np.ascontiguousarray(value[b].T).astype(f, copy=False),
                "wqT": np.ascontiguousarray(Wq[lo : lo + G].T).astype(f, copy=False),
                "wkT": np.ascontiguousarray(Wk[lo : lo + G].T).astype(f, copy=False),
                "wvT": np.ascontiguousarray(Wv[lo : lo + G].T).astype(f, copy=False),
                "woT": np.ascontiguousarray(Wo[:, lo : lo + G].T).astype(f, copy=False),
                "bqd": np.ascontiguousarray(bq[lo : lo + G].reshape(DC, 128).T).astype(np.float32),
                "bkd": np.ascontiguousarray(bk[lo : lo + G].reshape(DC, 128).T).astype(np.float32),
            }
        )
    return maps


def _numpy_reference(query, key, value, mask, Wq, bq, Wk, bk, Wv, bv, Wo, bo):
    """Exact numpy fallback (only used if mask is not all ones)."""
    q = (query @ Wq.T + bq).reshape(B, S, H, DH).transpose(0, 2, 1, 3)
    k = (key @ Wk.T + bk).reshape(B, S, H, DH).transpose(0, 2, 1, 3)
    v = (value @ Wv.T + bv).reshape(B, S, H, DH).transpose(0, 2, 1, 3)
    scores = np.einsum("bhqd,bhkd->bhqk", q, k) / np.sqrt(np.float32(E))
    m = mask[:, None, :, :]
    scores = np.where(m == 0, -np.inf, scores)
    scores -= scores.max(axis=-1, keepdims=True)
    p = np.exp(scores)
    p /= p.sum(axis=-1, keepdims=True)
    ctx = np.einsum("bhqk,bhkd->bhqd", p, v)
    concat = ctx.transpose(0, 2, 1, 3).reshape(B, S, E)
    return (concat @ Wo.T + bo).astype(np.float32)


def kernel(query, key, value, mask, Wq, bq, Wk, bk, Wv, bv, Wo, bo, _results=None):
    global _NC
    query = np.asarray(query, dtype=np.float32)
    key = np.asarray(key, dtype=np.float32)
    value = np.asarray(value, dtype=np.float32)
    mask = np.asarray(mask)
    Wq, bq = np.asarray(Wq, np.float32), np.asarray(bq, np.float32)
    Wk, bk = np.asarray(Wk, np.float32), np.asarray(bk, np.float32)
    Wv, bv = np.asarray(Wv, np.float32), np.asarray(bv, np.float32)
    Wo, bo = np.asarray(Wo, np.float32), np.asarray(bo, np.float32)

    if not np.all(mask == 1):
        return _numpy_reference(
            query, key, value, mask, Wq, bq, Wk, bk, Wv, bv, Wo, bo
        )

    if _NC is None:
        _NC = _build_program()
    in_maps = _prep_core_inputs(query, key, value, Wq, bq, Wk, bk, Wv, Wo)
    res = run_bass_kernel_spmd(_NC, in_maps, core_ids=list(range(8)))
    if _results is not None:
        _results.append(res)

    # host epilogue: sum the two head-group partials; bv commutes with softmax
    # (rows sum to 1) so its contribution is Wo @ bv, plus the output bias bo.
    extra = (Wo.astype(np.float64) @ bv.astype(np.float64) + bo.astype(np.float64)).astype(
        np.float32
    )
    out = np.empty((B, S, E), dtype=np.float32)
    for b in range(B):
        out[b] = (
            res.results[2 * b]["out"] + res.results[2 * b + 1]["out"]
        ).T + extra
    return out


# revision 9
# speedup vs baseline: 1.0364x; 1.0364x over previous
"""Multi-head attention (B=4, S=2048, D=1024, H=16) on 8 Trainium2 NeuronCores.

Sharding: core c handles batch c//2 and head-group c%2 (8 heads = 512 dims of
the per-head concat). Each core computes its q/k/v projections (tensor
parallel over heads), attention for its 8 heads, and a partial output
projection over its 512 concat dims; the host sums the two partials per batch.

Device dataflow (per core, all matmuls bf16 = full-rate PE with FWL weight
loads and background-buffer preload; fp32 PSUM accumulation):
  - prologue projects only kT[dc0] and qT[qt0][dc0]; attention starts ~10us
    in. All remaining projection work (v, kT dc1-3, qT chunks, out-proj)
    streams through a background generator queue interleaved between the
    attention matmuls, paced so the ACT-bound exp pipeline never starves.
  - scores computed transposed S^T[k, q]; the two heads of a pair are
    row-packed (tile rows 0/64) so they run concurrently on the PE.
  - exp(scale*s) is a single ACT op per k-chunk over both heads (the mask is
    all-ones per the problem spec; a numpy fallback handles anything else).
  - ctx^T = [V | 1]^T @ P^T accumulated over k-chunks; row 64 of the psum is
    the softmax denominator (flash-style deferred normalization).
  - normalization: reciprocal of the denominator row on DVE, partition
    broadcast on GpSimd, one DVE multiply per head into ctx^T.
  - out^T partial = ctx_cat^T chunks @ Wo^T slices, streamed to DRAM.

Host epilogue: out[b] = partial[2b] + partial[2b+1] + (Wo @ bv + bo); the
value bias commutes with softmax (rows sum to 1) so it is exact. The key bias
is softmax-invariant (constant per query) and is still applied on-device for
exactness; so is the query bias.
"""

import sys

sys.path.insert(0, "/opt/trn_rl_repo")

import numpy as np

import concourse.bacc as bacc
import concourse.mybir as mybir
import concourse.tile as tile
from concourse.bass_utils import run_bass_kernel_spmd

f32 = mybir.dt.float32
bf16 = mybir.dt.bfloat16
AF = mybir.ActivationFunctionType

B, S, E, H = 4, 2048, 1024, 16
DH = E // H  # 64
G = E // 2  # 512 dims per core (8 heads)
HL = H // 2  # heads per core
EC = E // 128  # 8 e-chunks (projection contraction)
DC = G // 128  # 4 head-pairs per core
QT = S // 512  # 4 q-tiles
KC = S // 128  # 16 k-chunks
GC = G // 128  # 4 chunks of the local concat dim (out-proj contraction)
SCALE = 1.0 / np.sqrt(np.float64(E))

_NC = None


def _build_program():
    nc = bacc.Bacc("TRN2", target_bir_lowering=False, debug=False, num_devices=8)

    xqT = nc.dram_tensor("xqT", [E, S], bf16, kind="ExternalInput").ap()
    xkT = nc.dram_tensor("xkT", [E, S], bf16, kind="ExternalInput").ap()
    xvT = nc.dram_tensor("xvT", [E, S], bf16, kind="ExternalInput").ap()
    wqT = nc.dram_tensor("wqT", [E, G], bf16, kind="ExternalInput").ap()
    wkT = nc.dram_tensor("wkT", [E, G], bf16, kind="ExternalInput").ap()
    wvT = nc.dram_tensor("wvT", [E, G], bf16, kind="ExternalInput").ap()
    woT = nc.dram_tensor("woT", [G, E], bf16, kind="ExternalInput").ap()
    bqd = nc.dram_tensor("bqd", [128, DC], f32, kind="ExternalInput").ap()
    bkd = nc.dram_tensor("bkd", [128, DC], f32, kind="ExternalInput").ap()
    out = nc.dram_tensor("out", [E, S], f32, kind="ExternalOutput").ap()  # transposed

    with tile.TileContext(nc) as tc:
        with (
            tc.tile_pool(name="weights", bufs=1) as wpool,
            tc.tile_pool(name="persist", bufs=1) as ppool,
            tc.tile_pool(name="xkpool", bufs=4) as xkpool,
            tc.tile_pool(name="xqstream", bufs=2) as xqstream,
            tc.tile_pool(name="vstream", bufs=2) as vstream,
            tc.tile_pool(name="qtile", bufs=2) as qpool,
            tc.tile_pool(name="ctxp", bufs=1) as ctxp,
            tc.tile_pool(name="exp", bufs=8) as epool,
            tc.tile_pool(name="norm", bufs=3) as npool,
            tc.tile_pool(name="outsb", bufs=4) as opool,
            tc.tile_pool(name="s_psum", bufs=2, space="PSUM") as s_psum,
            tc.tile_pool(name="c_psum", bufs=4, space="PSUM") as c_psum,
        ):
            kT_sb = ppool.tile([128, DC, S], bf16)
            v_sb = ppool.tile([128, KC, HL, DH + 1], bf16)
            ctxT_sb = ctxp.tile([128, DC, S], bf16)
            wq_sb = wpool.tile([128, EC, G], bf16)
            wk_sb = wpool.tile([128, EC, G], bf16)
            wv_sb = wpool.tile([128, EC, G], bf16)
            wo_sb = wpool.tile([128, GC, E], bf16)
            bq_sb = wpool.tile([128, DC], f32)
            bk_sb = wpool.tile([128, DC], f32)
            actwarm = wpool.tile([1, 8], f32)

            def xstream(pool, src, lo, eng, tag="xstream"):
                t = pool.tile([128, EC, 512], bf16, tag=tag)
                ap = src[:, lo : lo + 512].rearrange("(ec p) s -> p ec s", p=128)
                eng.dma_start(t[:], ap)
                return t

            # ones column for the denominator fusion (cols 0..DH-1 of each
            # head block are written by the v-projection before any read)
            nc.gpsimd.memset(v_sb[:, :, :, DH : DH + 1], 1.0)
            # pull the exp table load off the attention critical path
            nc.gpsimd.memset(actwarm[:], 0.0)
            nc.scalar.activation(actwarm[:], actwarm[:], AF.Exp)

            # -------- prologue DMAs, split across queues for latency --------
            # critical-path data first; rings share HBM bandwidth, so the
            # v-path rides the sync ring BEHIND the xk tiles, and wo is
            # deferred until mid-attention (emitted later on gpsimd).
            # sync queue: key-projection path, then value-projection path
            nc.sync.dma_start(wk_sb[:], wkT.rearrange("(ec p) g -> p ec g", p=128))
            xk_ts = [xstream(xkpool, xkT, st * 512, nc.sync, tag="xk") for st in range(QT)]
            nc.sync.dma_start(wv_sb[:], wvT.rearrange("(ec p) g -> p ec g", p=128))
            xv_ts = {0: xstream(vstream, xvT, 0, nc.sync, tag="xv")}
            # gpsimd queue: query-projection path (+ tiny biases)
            xq_ts = {0: xstream(xqstream, xqT, 0, nc.gpsimd, tag="xq")}
            nc.gpsimd.dma_start(wq_sb[:], wqT.rearrange("(ec p) g -> p ec g", p=128))
            nc.gpsimd.dma_start(bq_sb[:], bqd)
            nc.gpsimd.dma_start(bk_sb[:], bkd)

            # ---------------- projection generators ----------------
            def kproj_steps(dc):
                """kT projection for one head-pair, all 4 s-tiles."""
                for st in range(QT):
                    ps = c_psum.tile([128, 512], f32, tag="ctx", name=f"kp{dc}_{st}")
                    for ec in range(EC):
                        nc.tensor.matmul(
                            ps[:],
                            lhsT=wk_sb[:, ec, dc * 128 : (dc + 1) * 128],
                            rhs=xk_ts[st][:, ec, :],
                            start=(ec == 0),
                            stop=(ec == EC - 1),
                        )
                        if ec % 2 == 1:
                            yield
                    nc.vector.tensor_add(
                        out=kT_sb[:, dc, st * 512 : (st + 1) * 512],
                        in0=ps[:],
                        in1=bk_sb[:, dc : dc + 1].to_broadcast((128, 512)),
                    )

            def vproj_steps(sc):
                """v projection for one 128-key chunk (all local heads)."""
                sg, sci = sc // 4, sc % 4
                if sci == 0 and sg + 1 < QT and sg + 1 not in xv_ts:
                    xv_ts[sg + 1] = xstream(vstream, xvT, (sg + 1) * 512, nc.scalar, tag="xv")
                ps = c_psum.tile([128, 512], f32, tag="ctx", name=f"vp{sc}")
                for ec in range(EC):
                    nc.tensor.matmul(
                        ps[:, :G],
                        lhsT=xv_ts[sg][:, ec, sci * 128 : (sci + 1) * 128],
                        rhs=wv_sb[:, ec, :],
                        start=(ec == 0),
                        stop=(ec == EC - 1),
                    )
                    if ec % 2 == 1:
                        yield
                nc.vector.tensor_copy(
                    out=v_sb[:, sc, :, 0:DH],
                    in_=ps[:, :G].rearrange("p (h d) -> p h d", h=HL),
                )

            qT_ts = {}

            def qproj_steps(qt, dc):
                """one dc-chunk of the qT projection."""
                if dc == 0:
                    qT_ts[qt] = qpool.tile(
                        [128, DC, 512], bf16, tag="qT", name=f"qT{qt}"
                    )
                qT_t = qT_ts[qt]
                ps = c_psum.tile([128, 512], f32, tag="ctx", name=f"qp{qt}_{dc}")
                for ec in range(EC):
                    nc.tensor.matmul(
                        ps[:],
                        lhsT=wq_sb[:, ec, dc * 128 : (dc + 1) * 128],
                        rhs=xq_ts[qt][:, ec, :],
                        start=(ec == 0),
                        stop=(ec == EC - 1),
                    )
                    if ec % 2 == 1:
                        yield
                nc.vector.tensor_add(
                    out=qT_t[:, dc, :],
                    in0=ps[:],
                    in1=bq_sb[:, dc : dc + 1].to_broadcast((128, 512)),
                )

            def outproj_steps(st, ec):
                """one ec-chunk of the transposed output projection."""
                ps = c_psum.tile([128, 512], f32, tag="ctx", name=f"op{st}_{ec}")
                for gc in range(GC):
                    nc.tensor.matmul(
                        ps[:],
                        lhsT=wo_sb[:, gc, ec * 128 : (ec + 1) * 128],
                        rhs=ctxT_sb[:, gc, st * 512 : (st + 1) * 512],
                        start=(gc == 0),
                        stop=(gc == GC - 1),
                    )
                    if gc % 2 == 1:
                        yield
                o_sb = opool.tile([128, 512], f32, tag="osb")
                nc.vector.tensor_copy(out=o_sb[:], in_=ps[:])
                nc.sync.dma_start(
                    out[ec * 128 : (ec + 1) * 128, st * 512 : (st + 1) * 512],
                    o_sb[:],
                )

            # -------- background work queue (persists across hp/qt) --------
            bg = []  # list of (key, generator)
            done_keys = set()

            def drive(n=1):
                while n > 0 and bg:
                    try:
                        next(bg[0][1])
                        n -= 1
                    except StopIteration:
                        done_keys.add(bg[0][0])
                        bg.pop(0)

            def drain_until(key):
                while key not in done_keys and bg:
                    drive(1)

            # -------- prologue compute: kT[dc0] and qT[qt0][dc0] --------
            for _ in kproj_steps(0):
                pass
            done_keys.add(("kT", 0))
            for _ in qproj_steps(0, 0):
                pass
            done_keys.add(("qT", 0, 0))

            for sc in range(KC):
                bg.append((("v", sc), vproj_steps(sc)))
            for dc in range(1, DC):
                bg.append((("kT", dc), kproj_steps(dc)))
                bg.append((("qT", 0, dc), qproj_steps(0, dc)))

            # ---------------- attention main loop ----------------
            for qt in range(QT):
                q0 = qt * 512
                for hp in range(DC):
                    if hp == 0 and qt < QT - 1:
                        # pre-issue the next q-tile's activation DMA so the
                        # projection never waits on it when force-drained
                        xq_ts[qt + 1] = xstream(xqstream, xqT, (qt + 1) * 512, nc.sync, tag="xq")
                    if qt == 0 and hp == 2:
                        # wo is first needed by out-proj early in qt1; issuing
                        # it here keeps it clear of the prologue DMA window
                        nc.gpsimd.dma_start(
                            wo_sb[:], woT.rearrange("(gc p) e -> p gc e", p=128)
                        )
                    if qt < QT - 1:
                        bg.append((("qT", qt + 1, hp), qproj_steps(qt + 1, hp)))
                    if qt > 0:
                        bg.append((("op", qt - 1, 2 * hp), outproj_steps(qt - 1, 2 * hp)))
                        bg.append(
                            (("op", qt - 1, 2 * hp + 1), outproj_steps(qt - 1, 2 * hp + 1))
                        )
                    drain_until(("kT", hp))
                    drain_until(("qT", qt, hp))
                    qT_t = qT_ts[qt]
                    ctx0 = c_psum.tile([128, 512], f32, tag="ctx", name=f"c0_{qt}_{hp}")
                    ctx1 = c_psum.tile([128, 512], f32, tag="ctx", name=f"c1_{qt}_{hp}")
                    # software-pipelined: ctx(kc-1) and background work are
                    # emitted BEFORE the scores pair of kc so the scheduler
                    # keeps the two row-packed scores matmuls adjacent
                    pend = [None]

                    def ctx_pair(kc):
                        drain_until(("v", kc))
                        e = pend[0]
                        nc.tensor.matmul(
                            ctx0[0 : DH + 1, :],
                            lhsT=v_sb[:, kc, 2 * hp, :],
                            rhs=e[:, 0:512],
                            start=(kc == 0),
                            stop=(kc == KC - 1),
                        )
                        nc.tensor.matmul(
                            ctx1[0 : DH + 1, :],
                            lhsT=v_sb[:, kc, 2 * hp + 1, :],
                            rhs=e[:, 512:1024],
                            start=(kc == 0),
                            stop=(kc == KC - 1),
                        )

                    for kc in range(KC):
                        k0 = kc * 128
                        if kc > 0:
                            ctx_pair(kc - 1)
                        drive(2 if qt == 0 else 1)
                        sp = s_psum.tile([128, 1024], f32, tag="sp")
                        nc.tensor.matmul(
                            sp[:, 0:512],
                            lhsT=kT_sb[0:64, hp, k0 : k0 + 128],
                            rhs=qT_t[0:64, hp, :],
                            start=True,
                            stop=True,
                        )
                        nc.tensor.matmul(
                            sp[:, 512:1024],
                            lhsT=kT_sb[64:128, hp, k0 : k0 + 128],
                            rhs=qT_t[64:128, hp, :],
                            start=True,
                            stop=True,
                        )
                        e = epool.tile([128, 1024], bf16, tag="exp")
                        nc.scalar.activation(e[:], sp[:], AF.Exp, scale=float(SCALE))
                        pend[0] = e
                    ctx_pair(KC - 1)
                    # evacuate psum fast, then normalize in SBUF
                    for hq, cpsum in ((0, ctx0), (1, ctx1)):
                        pb = 64 * hq
                        qs = slice(q0, q0 + 512)
                        nc.vector.tensor_copy(
                            out=ctxT_sb[pb : pb + 64, hp, qs], in_=cpsum[0:DH, :]
                        )
                        den = npool.tile([1, 512], f32, tag="den")
                        nc.vector.tensor_copy(out=den[:], in_=cpsum[DH : DH + 1, :])
                        rec = npool.tile([1, 512], f32, tag="rec")
                        nc.vector.reciprocal_approx_fast(rec[:], den[:])
                        rb = npool.tile([128, 512], f32, tag="rb")
                        nc.gpsimd.partition_broadcast(rb[:], rec[:])
                        nc.vector.tensor_mul(
                            out=ctxT_sb[pb : pb + 64, hp, qs],
                            in0=ctxT_sb[pb : pb + 64, hp, qs],
                            in1=rb[pb : pb + 64, :],
                        )

            # drain any background leftovers, then the last q-tile's out-proj
            while bg:
                drive(1)
            for ec in range(EC):
                for _ in outproj_steps(QT - 1, ec):
                    pass

    nc.compile()
    return nc


def _prep_core_inputs(query, key, value, Wq, bq, Wk, bk, Wv, Wo):
    """Per-core input maps: core c -> batch c//2, head-group c%2."""
    import ml_dtypes

    f = ml_dtypes.bfloat16
    maps = []
    for c in range(8):
        b, g = c // 2, c % 2
        lo = g * G
        maps.append(
            {
                "xqT": np.ascontiguousarray(query[b].T).astype(f, copy=False),
                "xkT": np.ascontiguousarray(key[b].T).astype(f, copy=False),
                "xvT": np.ascontiguousarray(value[b].T).astype(f, copy=False),
                "wqT": np.ascontiguousarray(Wq[lo : lo + G].T).astype(f, copy=False),
                "wkT": np.ascontiguousarray(Wk[lo : lo + G].T).astype(f, copy=False),
                "wvT": np.ascontiguousarray(Wv[lo : lo + G].T).astype(f, copy=False),
                "woT": np.ascontiguousarray(Wo[:, lo : lo + G].T).astype(f, copy=False),
                "bqd": np.ascontiguousarray(bq[lo : lo + G].reshape(DC, 128).T).astype(np.float32),
                "bkd": np.ascontiguousarray(bk[lo : lo + G].reshape(DC, 128).T).astype(np.float32),
            }
        )
    return maps


def _numpy_reference(query, key, value, mask, Wq, bq, Wk, bk, Wv, bv, Wo, bo):
    """Exact numpy fallback (only used if mask is not all ones)."""
    q = (query @ Wq.T + bq).reshape(B, S, H, DH).transpose(0, 2, 1, 3)
    k = (key @ Wk.T + bk).reshape(B, S, H, DH).transpose(0, 2, 1, 3)
    v = (value @ Wv.T + bv).reshape(B, S, H, DH).transpose(0, 2, 1, 3)
    scores = np.einsum("bhqd,bhkd->bhqk", q, k) / np.sqrt(np.float32(E))
    m = mask[:, None, :, :]
    scores = np.where(m == 0, -np.inf, scores)
    scores -= scores.max(axis=-1, keepdims=True)
    p = np.exp(scores)
    p /= p.sum(axis=-1, keepdims=True)
    ctx = np.einsum("bhqk,bhkd->bhqd", p, v)
    concat = ctx.transpose(0, 2, 1, 3).reshape(B, S, E)
    return (concat @ Wo.T + bo).astype(np.float32)


def kernel(query, key, value, mask, Wq, bq, Wk, bk, Wv, bv, Wo, bo, _results=None):
    global _NC
    query = np.asarray(query, dtype=np.float32)
    key = np.asarray(key, dtype=np.float32)
    value = np.asarray(value, dtype=np.float32)
    mask = np.asarray(mask)
    Wq, bq = np.asarray(Wq, np.float32), np.asarray(bq, np.float32)
    Wk, bk = np.asarray(Wk, np.float32), np.asarray(bk, np.float32)
    Wv, bv = np.asarray(Wv, np.float32), np.asarray(bv, np.float32)
    Wo, bo = np.asarray(Wo, np.float32), np.asarray(bo, np.float32)

    if not np.all(mask == 1):
        return _numpy_reference(
            query, key, value, mask, Wq, bq, Wk, bk, Wv, bv, Wo, bo
        )

    if _NC is None:
        _NC = _build_program()
    in_maps = _prep_core_inputs(query, key, value, Wq, bq, Wk, bk, Wv, Wo)
    res = run_bass_kernel_spmd(_NC, in_maps, core_ids=list(range(8)))
    if _results is not None:
        _results.append(res)

    # host epilogue: sum the two head-group partials; bv commutes with softmax
    # (rows sum to 1) so its contribution is Wo @ bv, plus the output bias bo.
    extra = (Wo.astype(np.float64) @ bv.astype(np.float64) + bo.astype(np.float64)).astype(
        np.float32
    )
    out = np.empty((B, S, E), dtype=np.float32)
    for b in range(B):
        out[b] = (
            res.results[2 * b]["out"] + res.results[2 * b + 1]["out"]
        ).T + extra
    return out
